# revision 22
# baseline (speedup 1.0000x reference)
"""Trainium2 Bass kernel: grouped MoE expert MLP (nn_ExpertGroup).

Strategy: expert parallelism across 8 NeuronCores. Tokens are sorted by
expert; core e runs expert e's two GEMMs:
    h = relu(x_e @ w_up[e].T) ** 2      (bf16, like the CUDA reference)
    y = h @ w_down[e].T
The host does the (free) token scatter/gather, the bf16 casts, and the
weight transposes/swizzles so every device-side DMA moves >=512B
contiguous runs (line rate) and costs one ~0.65us engine issue.

Timing model (measured): ~6.5us fixed runtime prologue (event-semaphore
init + engine table loads + entry barrier), then a single just-in-time
ordered DMA stream on the sync HWDGE ring (~350 GB/s; splitting across
rings halves each stream's share), PE warmup matmuls bridging the HAM
clock ramp until the first GEMM1 operands land (~10.5us), a dense
109.2us bf16 PE stream (the roofline: 2 x 1024x1024x2048 MACs @ 16384
MAC/cycle, 2.4 GHz), then a short drain + single-barrier teardown.

Device layout (per core, cap = padded local token count, default 1024):
    xT  (D=1024, cap) bf16 x_e.T         -> SBUF [128, 8, cap]
    wuT swizzled [128, 16, 8, 128] bf16  (j-tile, d-tile, j-cols)
    wdT swizzled [128, 16, 1024]   bf16  (j-tile, output cols)
    GEMM1: psum[j,t] = sum_d wuT[:,j,d].T @ xT[:,d,c]   (h in [H, T] layout)
    DVE:   relu fp32 psum -> bf16, square -> hsq [128, 16, cap]
    GEMM2: psum[t,i] = sum_j hsq[j,t].T @ wdT[:,j,i]  (y in [T, D] layout)
    DVE:   cast fp32 psum -> bf16 y -> DMA out

Precision: bf16 everywhere (matches the reference's bf16 pipeline,
rel err ~5e-3). fp8 DoubleRow (2x PE) was evaluated and rejected: e4m3
quantization is ~2.7% rms per operand; uncompensated error is ~5.8e-2
(gate 2e-2) and full error-compensation costs 1.5x bf16 PE time.

Built on bacc.Bacc (not raw Bass): Bacc.compile() legalizes semaphore
waits to the TRN2 limit of one wait per instruction.
"""

import numpy as np
import ml_dtypes

import concourse.bass as bass
import concourse.mybir as mybir
import concourse.tile as tile
from concourse import bacc
from concourse.bass_utils import run_bass_kernel_spmd
from concourse.vector_clock import ScopedClock

T, D, H, E = 8192, 1024, 2048, 8
P = 128
N_CORES = 8
# GEMM1 token chunks. 512-wide chunks keep the c0 j-pass (13.8us) longer
# than the full w_up delivery (~11.4us), so the PE never starves; smaller
# first chunks start earlier but stall harder mid-pass (measured).
C_CHUNKS = [(0, 512), (512, 512)]
FD2 = 512  # GEMM2 moving free dim (one PSUM bank of fp32)
WARM_N = 26
n_d_host = D // P


def _slim_drain_and_barrier(self, tick_clock, wait_clock):
    """Replaces TileContext._drain_and_barrier: keep the load-bearing DMA
    drain (waits on all outstanding DMA completion semaphores) and one
    all-engine barrier, but skip the semaphore clear + second barrier
    (~1.5us). Each NEFF execution re-initializes semaphores in its own
    prologue, and run_bass_kernel_spmd executes the NEFF exactly once."""
    drain_inst = self.nc.sync.drain()
    wait_clock.add_sem_waits(
        drain_inst.ins, ScopedClock({None: tick_clock.global_clock})
    )
    popped = self.nc._tile_sem_poison_stack.pop()
    assert popped is self._sem_poison


tile.TileContext._drain_and_barrier = _slim_drain_and_barrier


def _ensure_axon_ntff_hook():
    """The container's `antenv` stub lacks `axon_hooks`; if BASS_TRACE=1 is
    set, run_bass_kernel_spmd would crash importing it. Recreate the tiny
    registry and register the ctypes NTFF hook so tracing works (and never
    let this best-effort setup break the kernel)."""
    try:
        import antenv.axon_hooks  # noqa: F401
        return
    except ImportError:
        pass
    try:
        import sys
        import types

        import antenv
        from trn_agent_boot.trn_boot import _ntff_profile_via_ctypes

        mod = types.ModuleType("antenv.axon_hooks")
        mod._hook = _ntff_profile_via_ctypes("/opt/axon/libaxon_pjrt.so")
        mod.set_axon_ntff_profile_hook = lambda h: setattr(mod, "_hook", h)
        mod.get_axon_ntff_profile_hook = lambda: mod._hook
        sys.modules["antenv.axon_hooks"] = mod
        antenv.axon_hooks = mod
    except Exception:
        pass


_ensure_axon_ntff_hook()

_PROGRAM_CACHE: dict[int, "bass.Bass"] = {}
LAST_RESULT = None  # BassKernelResults of the most recent run (for harness use)


def _build_program(cap: int) -> "bass.Bass":
    n_d = D // P       # 8  contraction tiles of GEMM1
    n_j = H // P       # 16 H partition tiles
    n_t = cap // P     # token partition tiles (GEMM2 output)
    n_ic = D // FD2    # GEMM2 output column chunks
    assert sum(w for _, w in C_CHUNKS) == cap
    bf16 = mybir.dt.bfloat16
    f32 = mybir.dt.float32

    nc = bacc.Bacc("TRN2", debug=False, num_devices=N_CORES)
    xT = nc.dram_tensor("xT", [D, cap], bf16, kind="ExternalInput")
    wuT = nc.dram_tensor("wuT", [P, n_j, n_d, P], bf16, kind="ExternalInput")
    wdT = nc.dram_tensor("wdT", [P, n_j, D], bf16, kind="ExternalInput")
    y = nc.dram_tensor("y", [cap, D], bf16, kind="ExternalOutput")

    xT3 = xT[:].rearrange("(po pi) f -> pi po f", pi=P)  # [128, 8, cap]
    y3 = y[:].rearrange("(po pi) f -> pi po f", pi=P)    # [128, n_t, 1024]

    with tile.TileContext(nc) as tc:
        with (
            tc.tile_pool(name="big", bufs=1) as big,
            tc.tile_pool(name="outp", bufs=4) as outp,
            tc.tile_pool(name="actp", bufs=4) as actp,
            tc.tile_pool(name="ps1", bufs=2, space="PSUM") as ps1p,
            tc.tile_pool(name="ps2", bufs=3, space="PSUM") as ps2p,
            tc.tile_pool(name="warmp", bufs=1, space="PSUM") as warmp,
        ):
            xT_sb = big.tile([P, n_d, cap], bf16)
            wuT_sb = big.tile([P, n_j, n_d, P], bf16)
            wdT_sb = big.tile([P, n_j, D], bf16)
            hsq_sb = big.tile([P, n_j, cap], bf16)

            # PE warm-up: dummy matmuls with no DMA dependencies run while
            # the input DMAs stream in, keeping the PE busy through the HAM
            # activity window so the real matmul stream starts at the full
            # 2.4 GHz clock instead of the cold 1.2 GHz.
            warm = big.tile([P, 256], bf16)
            nc.vector.memset(warm[:], 0.0)
            wps = warmp.tile([P, 256], f32, tag="warm")
            for _ in range(WARM_N):
                nc.tensor.matmul(wps, warm[:, 0:P], warm[:], start=True, stop=True)

            # Input DMAs: ONE ring (sync queue), just-in-time FIFO order at
            # full stream bandwidth (two rings split it; hoisting the issues
            # before the entry barrier was tried and reverted -- it delays
            # all other engines' barrier legs and the head is bandwidth-
            # bound anyway). The first x chunk and wu j-block gate the first
            # matmul; each later piece lands just before its consumption;
            # w_down (needed ~60us in) goes last.
            nc.scalar.dma_start(out=xT_sb[:, :, 0:512], in_=xT3[:, :, 0:512])
            nc.scalar.dma_start(out=xT_sb[:, :, 512:1024], in_=xT3[:, :, 512:1024])
            nc.sync.dma_start(out=wuT_sb[:, 0], in_=wuT[:, 0])
            nc.sync.dma_start(out=wuT_sb[:, 1], in_=wuT[:, 1])
            nc.sync.dma_start(out=wuT_sb[:, 2:4], in_=wuT[:, 2:4])
            nc.sync.dma_start(out=wuT_sb[:, 4:8], in_=wuT[:, 4:8])
            nc.sync.dma_start(out=wuT_sb[:, 8:12], in_=wuT[:, 8:12])
            nc.sync.dma_start(out=wuT_sb[:, 12:16], in_=wuT[:, 12:16])
            nc.sync.dma_start(out=wdT_sb[:], in_=wdT[:])

            # GEMM1 + relu^2: hsq[j, t] in SBUF, token-chunk outer
            for (t0, fw) in C_CHUNKS:
                assert fw == 512
                for j in range(n_j):
                    ps = ps2p.tile([P, fw], f32, tag="ps512")
                    for d in range(n_d):
                        nc.tensor.matmul(
                            ps,
                            wuT_sb[:, j, d],
                            xT_sb[:, d, t0:t0 + fw],
                            start=(d == 0),
                            stop=(d == n_d - 1),
                        )
                    hr = actp.tile([P, fw], bf16, tag=f"hr{fw}")
                    nc.vector.tensor_relu(out=hr, in_=ps)
                    nc.vector.tensor_mul(
                        out=hsq_sb[:, j, t0:t0 + fw], in0=hr, in1=hr
                    )

            # GEMM2: y[t, i] = sum_j hsq[j, t].T @ wdT[j, i]. The final
            # compute groups are narrowed (256/128/128) so the last
            # copy+DMA completion chain starts as early as possible.
            for t in range(n_t):
                pieces = [(ic * FD2, FD2) for ic in range(n_ic)]
                if t == n_t - 1:
                    pieces = pieces[:-1] + [
                        (D - 512, 256), (D - 256, 128), (D - 128, 128)]
                for (i0, fw) in pieces:
                    # narrow final pieces share one [P, 256] psum tag so the
                    # PSUM footprint stays within the 8 banks
                    if fw == 512:
                        ps = ps2p.tile([P, 512], f32, tag="ps512")
                    else:
                        ps = ps1p.tile([P, 256], f32, tag="ps256")
                    for j in range(n_j):
                        nc.tensor.matmul(
                            ps[:, 0:fw],
                            hsq_sb[:, j, t * P:(t + 1) * P],
                            wdT_sb[:, j, i0:i0 + fw],
                            start=(j == 0),
                            stop=(j == n_j - 1),
                        )
                    yt = outp.tile([P, fw], bf16, tag=f"yt{fw}")
                    nc.vector.tensor_copy(out=yt, in_=ps[:, 0:fw])
                    nc.sync.dma_start(out=y3[:, t, i0:i0 + fw], in_=yt)

    nc.compile()
    return nc


def _get_program(cap: int) -> "bass.Bass":
    nc = _PROGRAM_CACHE.get(cap)
    if nc is None:
        nc = _build_program(cap)
        _PROGRAM_CACHE[cap] = nc
    return nc


CAP = 1024  # tokens per core per round (the uniform T/E split = one round)


def _swizzle_wu(wu8):
    """[H, D] bf16 -> [128, 16, 8, 128] with wuT[p, j, d, t] =
    wu8[j*128 + t, d*128 + p]."""
    a = wu8.reshape(H // P, P, n_d_host, P)  # (j, t, d, p)
    return np.ascontiguousarray(a.transpose(3, 0, 2, 1))


def _swizzle_wd(wd8):
    """[D, H] bf16 -> [128, 16, 1024] with wdT[p, j, i] =
    wd8[i, j*128 + p]."""
    a = wd8.T.reshape(H // P, P, D)  # (j, p, i)
    return np.ascontiguousarray(a.transpose(1, 0, 2))


def kernel(x, num_tokens_per_expert, w_up, w_down, _trace=False):
    global LAST_RESULT
    bf = ml_dtypes.bfloat16
    x = np.asarray(x)
    counts = np.asarray(num_tokens_per_expert).astype(np.int64)
    w_up = np.asarray(w_up)
    w_down = np.asarray(w_down)
    n_tok = x.shape[0]
    assert counts.shape == (E,) and int(counts.sum()) == n_tok
    offsets = np.zeros(E, dtype=np.int64)
    offsets[1:] = np.cumsum(counts)[:-1]

    nc = _get_program(CAP)

    # Work list: split each expert's contiguous token segment into slots of
    # <= CAP tokens; process 8 slots per SPMD round. The uniform T/E = 1024
    # split is exactly one round of 8 slots.
    slots = []
    for e in range(E):
        cnt, off = int(counts[e]), int(offsets[e])
        for s in range(0, cnt, CAP):
            slots.append((e, off + s, min(CAP, cnt - s)))

    wuT_cache = {}
    wdT_cache = {}

    def expert_weights(e):
        if e not in wuT_cache:
            wuT_cache[e] = _swizzle_wu(w_up[e].astype(bf))
            wdT_cache[e] = _swizzle_wd(w_down[e].astype(bf))
        return wuT_cache[e], wdT_cache[e]

    out = np.zeros((n_tok, D), dtype=x.dtype)
    zero_map = None
    for r0 in range(0, len(slots), N_CORES):
        round_slots = slots[r0:r0 + N_CORES]
        in_maps = []
        for e, off, cnt in round_slots:
            xs = np.zeros((CAP, D), dtype=bf)
            xs[:cnt] = x[off:off + cnt].astype(bf)
            wuT, wdT = expert_weights(e)
            in_maps.append({
                "xT": np.ascontiguousarray(xs.T), "wuT": wuT, "wdT": wdT,
            })
        while len(in_maps) < N_CORES:  # idle cores in the last round
            if zero_map is None:
                zero_map = {
                    "xT": np.zeros((D, CAP), dtype=bf),
                    "wuT": np.zeros((P, H // P, n_d_host, P), dtype=bf),
                    "wdT": np.zeros((P, H // P, D), dtype=bf),
                }
            in_maps.append(zero_map)

        res = run_bass_kernel_spmd(
            nc, in_maps, core_ids=list(range(N_CORES)), trace=_trace
        )
        LAST_RESULT = res
        for i, (e, off, cnt) in enumerate(round_slots):
            out[off:off + cnt] = res.results[i]["y"][:cnt].astype(x.dtype)
    return out


# revision 23
# speedup vs baseline: 1.0297x; 1.0297x over previous
"""Trainium2 Bass kernel: grouped MoE expert MLP (nn_ExpertGroup).

Strategy: expert parallelism across 8 NeuronCores. Tokens are sorted by
expert; core e runs expert e's two GEMMs:
    h = relu(x_e @ w_up[e].T) ** 2      (bf16, like the CUDA reference)
    y = h @ w_down[e].T
The host does the (free) token scatter/gather, the bf16 casts, and the
weight transposes/swizzles so every device-side DMA moves >=512B
contiguous runs (line rate) and costs one ~0.65us engine issue.

Timing model (measured): ~6.5us fixed runtime prologue (event-semaphore
init + engine table loads + entry barrier), then a single just-in-time
ordered DMA stream on the sync HWDGE ring (~350 GB/s; splitting across
rings halves each stream's share), PE warmup matmuls bridging the HAM
clock ramp until the first GEMM1 operands land (~10.5us), a dense
109.2us bf16 PE stream (the roofline: 2 x 1024x1024x2048 MACs @ 16384
MAC/cycle, 2.4 GHz), then a short drain + single-barrier teardown.

Device layout (per core, cap = padded local token count, default 1024):
    xT  (D=1024, cap) bf16 x_e.T         -> SBUF [128, 8, cap]
    wuT swizzled [128, 16, 8, 128] bf16  (j-tile, d-tile, j-cols)
    wdT swizzled [128, 16, 1024]   bf16  (j-tile, output cols)
    GEMM1: psum[j,t] = sum_d wuT[:,j,d].T @ xT[:,d,c]   (h in [H, T] layout)
    DVE:   relu fp32 psum -> bf16, square -> hsq [128, 16, cap]
    GEMM2: psum[t,i] = sum_j hsq[j,t].T @ wdT[:,j,i]  (y in [T, D] layout)
    DVE:   cast fp32 psum -> bf16 y -> DMA out

Precision: bf16 everywhere (matches the reference's bf16 pipeline,
rel err ~5e-3). fp8 DoubleRow (2x PE) was evaluated and rejected: e4m3
quantization is ~2.7% rms per operand; uncompensated error is ~5.8e-2
(gate 2e-2) and full error-compensation costs 1.5x bf16 PE time.

Built on bacc.Bacc (not raw Bass): Bacc.compile() legalizes semaphore
waits to the TRN2 limit of one wait per instruction.
"""

import numpy as np
import ml_dtypes

import concourse.bass as bass
import concourse.mybir as mybir
import concourse.tile as tile
from concourse import bacc
from concourse.bass_utils import run_bass_kernel_spmd
from concourse.vector_clock import ScopedClock

T, D, H, E = 8192, 1024, 2048, 8
P = 128
N_CORES = 8
# GEMM1 token chunks. 512-wide chunks keep the c0 j-pass (13.8us) longer
# than the full w_up delivery (~11.4us), so the PE never starves; smaller
# first chunks start earlier but stall harder mid-pass (measured).
C_CHUNKS = [(0, 512), (512, 512)]
FD2 = 512  # GEMM2 moving free dim (one PSUM bank of fp32)
WARM_N = 30
n_d_host = D // P


def _slim_drain_and_barrier(self, tick_clock, wait_clock):
    """Replaces TileContext._drain_and_barrier: keep the load-bearing DMA
    drain (waits on all outstanding DMA completion semaphores) and one
    all-engine barrier, but skip the semaphore clear + second barrier
    (~1.5us). Each NEFF execution re-initializes semaphores in its own
    prologue, and run_bass_kernel_spmd executes the NEFF exactly once."""
    drain_inst = self.nc.sync.drain()
    wait_clock.add_sem_waits(
        drain_inst.ins, ScopedClock({None: tick_clock.global_clock})
    )
    popped = self.nc._tile_sem_poison_stack.pop()
    assert popped is self._sem_poison


tile.TileContext._drain_and_barrier = _slim_drain_and_barrier


def _ensure_axon_ntff_hook():
    """The container's `antenv` stub lacks `axon_hooks`; if BASS_TRACE=1 is
    set, run_bass_kernel_spmd would crash importing it. Recreate the tiny
    registry and register the ctypes NTFF hook so tracing works (and never
    let this best-effort setup break the kernel)."""
    try:
        import antenv.axon_hooks  # noqa: F401
        return
    except ImportError:
        pass
    try:
        import sys
        import types

        import antenv
        from trn_agent_boot.trn_boot import _ntff_profile_via_ctypes

        mod = types.ModuleType("antenv.axon_hooks")
        mod._hook = _ntff_profile_via_ctypes("/opt/axon/libaxon_pjrt.so")
        mod.set_axon_ntff_profile_hook = lambda h: setattr(mod, "_hook", h)
        mod.get_axon_ntff_profile_hook = lambda: mod._hook
        sys.modules["antenv.axon_hooks"] = mod
        antenv.axon_hooks = mod
    except Exception:
        pass


_ensure_axon_ntff_hook()

_PROGRAM_CACHE: dict[int, "bass.Bass"] = {}
LAST_RESULT = None  # BassKernelResults of the most recent run (for harness use)


def _build_program(cap: int) -> "bass.Bass":
    n_d = D // P       # 8  contraction tiles of GEMM1
    n_j = H // P       # 16 H partition tiles
    n_t = cap // P     # token partition tiles (GEMM2 output)
    n_ic = D // FD2    # GEMM2 output column chunks
    assert sum(w for _, w in C_CHUNKS) == cap
    bf16 = mybir.dt.bfloat16
    f32 = mybir.dt.float32

    nc = bacc.Bacc("TRN2", debug=False, num_devices=N_CORES)
    xT = nc.dram_tensor("xT", [D, cap], bf16, kind="ExternalInput")
    wuT = nc.dram_tensor("wuT", [P, n_j, n_d, P], bf16, kind="ExternalInput")
    wdT = nc.dram_tensor("wdT", [P, n_j, D], bf16, kind="ExternalInput")
    y = nc.dram_tensor("y", [cap, D], bf16, kind="ExternalOutput")

    xT3 = xT[:].rearrange("(po pi) f -> pi po f", pi=P)  # [128, 8, cap]
    y3 = y[:].rearrange("(po pi) f -> pi po f", pi=P)    # [128, n_t, 1024]

    with tile.TileContext(nc) as tc:
        with (
            tc.tile_pool(name="big", bufs=1) as big,
            tc.tile_pool(name="outp", bufs=4) as outp,
            tc.tile_pool(name="actp", bufs=4) as actp,
            tc.tile_pool(name="ps1", bufs=2, space="PSUM") as ps1p,
            tc.tile_pool(name="ps2", bufs=3, space="PSUM") as ps2p,
            tc.tile_pool(name="warmp", bufs=1, space="PSUM") as warmp,
        ):
            xT_sb = big.tile([P, n_d, cap], bf16)
            wuT_sb = big.tile([P, n_j, n_d, P], bf16)
            wdT_sb = big.tile([P, n_j, D], bf16)
            hsq_sb = big.tile([P, n_j, cap], bf16)

            # PE warm-up: dummy matmuls with no DMA dependencies run while
            # the input DMAs stream in, keeping the PE busy through the HAM
            # activity window so the real matmul stream starts at the full
            # 2.4 GHz clock instead of the cold 1.2 GHz.
            warm = big.tile([P, 256], bf16)
            nc.vector.memset(warm[:], 0.0)
            wps = warmp.tile([P, 256], f32, tag="warm")
            for _ in range(WARM_N):
                nc.tensor.matmul(wps, warm[:, 0:P], warm[:], start=True, stop=True)

            # Input DMAs: ONE ring (sync queue), just-in-time FIFO order at
            # full stream bandwidth (two rings split it; hoisting the issues
            # before the entry barrier was tried and reverted -- it delays
            # all other engines' barrier legs and the head is bandwidth-
            # bound anyway). The first x chunk and wu j-block gate the first
            # matmul; each later piece lands just before its consumption;
            # w_down (needed ~60us in) goes last.
            nc.sync.dma_start(out=xT_sb[:, :, 0:512], in_=xT3[:, :, 0:512])
            nc.sync.dma_start(out=wuT_sb[:, 0], in_=wuT[:, 0])
            nc.sync.dma_start(out=wuT_sb[:, 1], in_=wuT[:, 1])
            nc.sync.dma_start(out=wuT_sb[:, 2:4], in_=wuT[:, 2:4])
            nc.sync.dma_start(out=wuT_sb[:, 4:8], in_=wuT[:, 4:8])
            nc.sync.dma_start(out=wuT_sb[:, 8:12], in_=wuT[:, 8:12])
            nc.sync.dma_start(out=wuT_sb[:, 12:16], in_=wuT[:, 12:16])
            nc.sync.dma_start(out=xT_sb[:, :, 512:1024], in_=xT3[:, :, 512:1024])
            nc.sync.dma_start(out=wdT_sb[:], in_=wdT[:])

            # GEMM1 + relu^2: hsq[j, t] in SBUF, token-chunk outer
            for (t0, fw) in C_CHUNKS:
                assert fw == 512
                for j in range(n_j):
                    ps = ps2p.tile([P, fw], f32, tag="ps512")
                    for d in range(n_d):
                        nc.tensor.matmul(
                            ps,
                            wuT_sb[:, j, d],
                            xT_sb[:, d, t0:t0 + fw],
                            start=(d == 0),
                            stop=(d == n_d - 1),
                        )
                    hr = actp.tile([P, fw], bf16, tag=f"hr{fw}")
                    nc.vector.tensor_relu(out=hr, in_=ps)
                    nc.vector.tensor_mul(
                        out=hsq_sb[:, j, t0:t0 + fw], in0=hr, in1=hr
                    )

            # GEMM2: y[t, i] = sum_j hsq[j, t].T @ wdT[j, i]. The final
            # compute groups are narrowed (256/128/128) so the last
            # copy+DMA completion chain starts as early as possible.
            for t in range(n_t):
                pieces = [(ic * FD2, FD2) for ic in range(n_ic)]
                if t == n_t - 1:
                    pieces = pieces[:-1] + [
                        (D - 512, 256), (D - 256, 128), (D - 128, 128)]
                for (i0, fw) in pieces:
                    # narrow final pieces share one [P, 256] psum tag so the
                    # PSUM footprint stays within the 8 banks
                    if fw == 512:
                        ps = ps2p.tile([P, 512], f32, tag="ps512")
                    else:
                        ps = ps1p.tile([P, 256], f32, tag="ps256")
                    for j in range(n_j):
                        nc.tensor.matmul(
                            ps[:, 0:fw],
                            hsq_sb[:, j, t * P:(t + 1) * P],
                            wdT_sb[:, j, i0:i0 + fw],
                            start=(j == 0),
                            stop=(j == n_j - 1),
                        )
                    yt = outp.tile([P, fw], bf16, tag=f"yt{fw}")
                    nc.vector.tensor_copy(out=yt, in_=ps[:, 0:fw])
                    nc.sync.dma_start(out=y3[:, t, i0:i0 + fw], in_=yt)

    nc.compile()
    return nc


def _get_program(cap: int) -> "bass.Bass":
    nc = _PROGRAM_CACHE.get(cap)
    if nc is None:
        nc = _build_program(cap)
        _PROGRAM_CACHE[cap] = nc
    return nc


CAP = 1024  # tokens per core per round (the uniform T/E split = one round)


def _swizzle_wu(wu8):
    """[H, D] bf16 -> [128, 16, 8, 128] with wuT[p, j, d, t] =
    wu8[j*128 + t, d*128 + p]."""
    a = wu8.reshape(H // P, P, n_d_host, P)  # (j, t, d, p)
    return np.ascontiguousarray(a.transpose(3, 0, 2, 1))


def _swizzle_wd(wd8):
    """[D, H] bf16 -> [128, 16, 1024] with wdT[p, j, i] =
    wd8[i, j*128 + p]."""
    a = wd8.T.reshape(H // P, P, D)  # (j, p, i)
    return np.ascontiguousarray(a.transpose(1, 0, 2))


def kernel(x, num_tokens_per_expert, w_up, w_down, _trace=False):
    global LAST_RESULT
    bf = ml_dtypes.bfloat16
    x = np.asarray(x)
    counts = np.asarray(num_tokens_per_expert).astype(np.int64)
    w_up = np.asarray(w_up)
    w_down = np.asarray(w_down)
    n_tok = x.shape[0]
    assert counts.shape == (E,) and int(counts.sum()) == n_tok
    offsets = np.zeros(E, dtype=np.int64)
    offsets[1:] = np.cumsum(counts)[:-1]

    nc = _get_program(CAP)

    # Work list: split each expert's contiguous token segment into slots of
    # <= CAP tokens; process 8 slots per SPMD round. The uniform T/E = 1024
    # split is exactly one round of 8 slots.
    slots = []
    for e in range(E):
        cnt, off = int(counts[e]), int(offsets[e])
        for s in range(0, cnt, CAP):
            slots.append((e, off + s, min(CAP, cnt - s)))

    wuT_cache = {}
    wdT_cache = {}

    def expert_weights(e):
        if e not in wuT_cache:
            wuT_cache[e] = _swizzle_wu(w_up[e].astype(bf))
            wdT_cache[e] = _swizzle_wd(w_down[e].astype(bf))
        return wuT_cache[e], wdT_cache[e]

    out = np.zeros((n_tok, D), dtype=x.dtype)
    zero_map = None
    for r0 in range(0, len(slots), N_CORES):
        round_slots = slots[r0:r0 + N_CORES]
        in_maps = []
        for e, off, cnt in round_slots:
            xs = np.zeros((CAP, D), dtype=bf)
            xs[:cnt] = x[off:off + cnt].astype(bf)
            wuT, wdT = expert_weights(e)
            in_maps.append({
                "xT": np.ascontiguousarray(xs.T), "wuT": wuT, "wdT": wdT,
            })
        while len(in_maps) < N_CORES:  # idle cores in the last round
            if zero_map is None:
                zero_map = {
                    "xT": np.zeros((D, CAP), dtype=bf),
                    "wuT": np.zeros((P, H // P, n_d_host, P), dtype=bf),
                    "wdT": np.zeros((P, H // P, D), dtype=bf),
                }
            in_maps.append(zero_map)

        res = run_bass_kernel_spmd(
            nc, in_maps, core_ids=list(range(N_CORES)), trace=_trace
        )
        LAST_RESULT = res
        for i, (e, off, cnt) in enumerate(round_slots):
            out[off:off + cnt] = res.results[i]["y"][:cnt].astype(x.dtype)
    return out


# revision 24
# speedup vs baseline: 1.0310x; 1.0013x over previous
"""Trainium2 Bass kernel: grouped MoE expert MLP (nn_ExpertGroup).

Strategy: expert parallelism across 8 NeuronCores. Tokens are sorted by
expert; core e runs expert e's two GEMMs:
    h = relu(x_e @ w_up[e].T) ** 2      (bf16, like the CUDA reference)
    y = h @ w_down[e].T
The host does the (free) token scatter/gather, the bf16 casts, and the
weight transposes/swizzles so every device-side DMA moves >=512B
contiguous runs (line rate) and costs one ~0.65us engine issue.

Timing model (measured): ~6.5us fixed runtime prologue (event-semaphore
init + engine table loads + entry barrier), then a single just-in-time
ordered DMA stream on the sync HWDGE ring (~350 GB/s; splitting across
rings halves each stream's share), PE warmup matmuls bridging the HAM
clock ramp until the first GEMM1 operands land (~10.5us), a dense
109.2us bf16 PE stream (the roofline: 2 x 1024x1024x2048 MACs @ 16384
MAC/cycle, 2.4 GHz), then a short drain + single-barrier teardown.

Device layout (per core, cap = padded local token count, default 1024):
    xT  (D=1024, cap) bf16 x_e.T         -> SBUF [128, 8, cap]
    wuT swizzled [128, 16, 8, 128] bf16  (j-tile, d-tile, j-cols)
    wdT swizzled [128, 16, 1024]   bf16  (j-tile, output cols)
    GEMM1: psum[j,t] = sum_d wuT[:,j,d].T @ xT[:,d,c]   (h in [H, T] layout)
    DVE:   relu fp32 psum -> bf16, square -> hsq [128, 16, cap]
    GEMM2: psum[t,i] = sum_j hsq[j,t].T @ wdT[:,j,i]  (y in [T, D] layout)
    DVE:   cast fp32 psum -> bf16 y -> DMA out

Precision: bf16 everywhere (matches the reference's bf16 pipeline,
rel err ~5e-3). fp8 DoubleRow (2x PE) was evaluated and rejected: e4m3
quantization is ~2.7% rms per operand; uncompensated error is ~5.8e-2
(gate 2e-2) and full error-compensation costs 1.5x bf16 PE time.

Built on bacc.Bacc (not raw Bass): Bacc.compile() legalizes semaphore
waits to the TRN2 limit of one wait per instruction.
"""

import numpy as np
import ml_dtypes

import concourse.bass as bass
import concourse.mybir as mybir
import concourse.tile as tile
from concourse import bacc
from concourse.bass_utils import run_bass_kernel_spmd
from concourse.vector_clock import ScopedClock

T, D, H, E = 8192, 1024, 2048, 8
P = 128
N_CORES = 8
# GEMM1 token chunks. 512-wide chunks keep the c0 j-pass (13.8us) longer
# than the full w_up delivery (~11.4us), so the PE never starves; smaller
# first chunks start earlier but stall harder mid-pass (measured).
C_CHUNKS = [(0, 512), (512, 512)]
FD2 = 512  # GEMM2 moving free dim (one PSUM bank of fp32)
WARM_N = 30
n_d_host = D // P


def _slim_drain_and_barrier(self, tick_clock, wait_clock):
    """Replaces TileContext._drain_and_barrier: keep the load-bearing DMA
    drain (waits on all outstanding DMA completion semaphores) and one
    all-engine barrier, but skip the semaphore clear + second barrier
    (~1.5us). Each NEFF execution re-initializes semaphores in its own
    prologue, and run_bass_kernel_spmd executes the NEFF exactly once."""
    drain_inst = self.nc.sync.drain()
    wait_clock.add_sem_waits(
        drain_inst.ins, ScopedClock({None: tick_clock.global_clock})
    )
    popped = self.nc._tile_sem_poison_stack.pop()
    assert popped is self._sem_poison


tile.TileContext._drain_and_barrier = _slim_drain_and_barrier


def _ensure_axon_ntff_hook():
    """The container's `antenv` stub lacks `axon_hooks`; if BASS_TRACE=1 is
    set, run_bass_kernel_spmd would crash importing it. Recreate the tiny
    registry and register the ctypes NTFF hook so tracing works (and never
    let this best-effort setup break the kernel)."""
    try:
        import antenv.axon_hooks  # noqa: F401
        return
    except ImportError:
        pass
    try:
        import sys
        import types

        import antenv
        from trn_agent_boot.trn_boot import _ntff_profile_via_ctypes

        mod = types.ModuleType("antenv.axon_hooks")
        mod._hook = _ntff_profile_via_ctypes("/opt/axon/libaxon_pjrt.so")
        mod.set_axon_ntff_profile_hook = lambda h: setattr(mod, "_hook", h)
        mod.get_axon_ntff_profile_hook = lambda: mod._hook
        sys.modules["antenv.axon_hooks"] = mod
        antenv.axon_hooks = mod
    except Exception:
        pass


_ensure_axon_ntff_hook()

_PROGRAM_CACHE: dict[int, "bass.Bass"] = {}
LAST_RESULT = None  # BassKernelResults of the most recent run (for harness use)


def _build_program(cap: int) -> "bass.Bass":
    n_d = D // P       # 8  contraction tiles of GEMM1
    n_j = H // P       # 16 H partition tiles
    n_t = cap // P     # token partition tiles (GEMM2 output)
    n_ic = D // FD2    # GEMM2 output column chunks
    assert sum(w for _, w in C_CHUNKS) == cap
    bf16 = mybir.dt.bfloat16
    f32 = mybir.dt.float32

    nc = bacc.Bacc("TRN2", debug=False, num_devices=N_CORES)
    # Raw SBUF scratch for the PE warmup. Its contents are irrelevant (the
    # warmup PSUM tile is never read, NaNs included), so no memset and no
    # cross-engine dependency: the PE can start warming the HAM clock the
    # moment it clears the entry barrier.
    warm_t = nc.alloc_sbuf_tensor("warm_scratch", [P, 256], bf16)
    xT = nc.dram_tensor("xT", [D, cap], bf16, kind="ExternalInput")
    wuT = nc.dram_tensor("wuT", [P, n_j, n_d, P], bf16, kind="ExternalInput")
    wdT = nc.dram_tensor("wdT", [P, n_j, D], bf16, kind="ExternalInput")
    y = nc.dram_tensor("y", [cap, D], bf16, kind="ExternalOutput")

    xT3 = xT[:].rearrange("(po pi) f -> pi po f", pi=P)  # [128, 8, cap]
    y3 = y[:].rearrange("(po pi) f -> pi po f", pi=P)    # [128, n_t, 1024]

    with tile.TileContext(nc) as tc:
        with (
            tc.tile_pool(name="big", bufs=1) as big,
            tc.tile_pool(name="outp", bufs=4) as outp,
            tc.tile_pool(name="actp", bufs=4) as actp,
            tc.tile_pool(name="ps1", bufs=2, space="PSUM") as ps1p,
            tc.tile_pool(name="ps2", bufs=3, space="PSUM") as ps2p,
            tc.tile_pool(name="warmp", bufs=1, space="PSUM") as warmp,
        ):
            xT_sb = big.tile([P, n_d, cap], bf16)
            wuT_sb = big.tile([P, n_j, n_d, P], bf16)
            wdT_sb = big.tile([P, n_j, D], bf16)
            hsq_sb = big.tile([P, n_j, cap], bf16)

            # PE warm-up: dummy matmuls with no DMA dependencies run while
            # the input DMAs stream in, keeping the PE busy through the HAM
            # activity window so the real matmul stream starts at the full
            # 2.4 GHz clock instead of the cold 1.2 GHz.
            wps = warmp.tile([P, 256], f32, tag="warm")
            for _ in range(WARM_N):
                nc.tensor.matmul(
                    wps, warm_t[:, 0:P], warm_t[:], start=True, stop=True)

            # Input DMAs: ONE ring (sync queue), just-in-time FIFO order at
            # full stream bandwidth (two rings split it; hoisting the issues
            # before the entry barrier was tried and reverted -- it delays
            # all other engines' barrier legs and the head is bandwidth-
            # bound anyway). The first x chunk and wu j-block gate the first
            # matmul; each later piece lands just before its consumption;
            # w_down (needed ~60us in) goes last.
            nc.sync.dma_start(out=xT_sb[:, :, 0:512], in_=xT3[:, :, 0:512])
            nc.sync.dma_start(out=wuT_sb[:, 0], in_=wuT[:, 0])
            nc.sync.dma_start(out=wuT_sb[:, 1], in_=wuT[:, 1])
            nc.sync.dma_start(out=wuT_sb[:, 2:4], in_=wuT[:, 2:4])
            nc.sync.dma_start(out=wuT_sb[:, 4:8], in_=wuT[:, 4:8])
            nc.sync.dma_start(out=wuT_sb[:, 8:12], in_=wuT[:, 8:12])
            nc.sync.dma_start(out=wuT_sb[:, 12:16], in_=wuT[:, 12:16])
            nc.sync.dma_start(out=xT_sb[:, :, 512:1024], in_=xT3[:, :, 512:1024])
            nc.sync.dma_start(out=wdT_sb[:], in_=wdT[:])

            # GEMM1 + relu^2: hsq[j, t] in SBUF, token-chunk outer
            for (t0, fw) in C_CHUNKS:
                assert fw == 512
                for j in range(n_j):
                    ps = ps2p.tile([P, fw], f32, tag="ps512")
                    for d in range(n_d):
                        nc.tensor.matmul(
                            ps,
                            wuT_sb[:, j, d],
                            xT_sb[:, d, t0:t0 + fw],
                            start=(d == 0),
                            stop=(d == n_d - 1),
                        )
                    hr = actp.tile([P, fw], bf16, tag=f"hr{fw}")
                    nc.vector.tensor_relu(out=hr, in_=ps)
                    nc.vector.tensor_mul(
                        out=hsq_sb[:, j, t0:t0 + fw], in0=hr, in1=hr
                    )

            # GEMM2: y[t, i] = sum_j hsq[j, t].T @ wdT[j, i]. The final
            # compute groups are narrowed (256/128/128) so the last
            # copy+DMA completion chain starts as early as possible.
            for t in range(n_t):
                pieces = [(ic * FD2, FD2) for ic in range(n_ic)]
                if t == n_t - 1:
                    pieces = pieces[:-1] + [
                        (D - 512, 256), (D - 256, 128), (D - 128, 128)]
                for (i0, fw) in pieces:
                    # narrow final pieces share one [P, 256] psum tag so the
                    # PSUM footprint stays within the 8 banks
                    if fw == 512:
                        ps = ps2p.tile([P, 512], f32, tag="ps512")
                    else:
                        ps = ps1p.tile([P, 256], f32, tag="ps256")
                    for j in range(n_j):
                        nc.tensor.matmul(
                            ps[:, 0:fw],
                            hsq_sb[:, j, t * P:(t + 1) * P],
                            wdT_sb[:, j, i0:i0 + fw],
                            start=(j == 0),
                            stop=(j == n_j - 1),
                        )
                    yt = outp.tile([P, fw], bf16, tag=f"yt{fw}")
                    nc.vector.tensor_copy(out=yt, in_=ps[:, 0:fw])
                    nc.sync.dma_start(out=y3[:, t, i0:i0 + fw], in_=yt)

    nc.compile()
    return nc


def _get_program(cap: int) -> "bass.Bass":
    nc = _PROGRAM_CACHE.get(cap)
    if nc is None:
        nc = _build_program(cap)
        _PROGRAM_CACHE[cap] = nc
    return nc


CAP = 1024  # tokens per core per round (the uniform T/E split = one round)


def _swizzle_wu(wu8):
    """[H, D] bf16 -> [128, 16, 8, 128] with wuT[p, j, d, t] =
    wu8[j*128 + t, d*128 + p]."""
    a = wu8.reshape(H // P, P, n_d_host, P)  # (j, t, d, p)
    return np.ascontiguousarray(a.transpose(3, 0, 2, 1))


def _swizzle_wd(wd8):
    """[D, H] bf16 -> [128, 16, 1024] with wdT[p, j, i] =
    wd8[i, j*128 + p]."""
    a = wd8.T.reshape(H // P, P, D)  # (j, p, i)
    return np.ascontiguousarray(a.transpose(1, 0, 2))


def kernel(x, num_tokens_per_expert, w_up, w_down, _trace=False):
    global LAST_RESULT
    bf = ml_dtypes.bfloat16
    x = np.asarray(x)
    counts = np.asarray(num_tokens_per_expert).astype(np.int64)
    w_up = np.asarray(w_up)
    w_down = np.asarray(w_down)
    n_tok = x.shape[0]
    assert counts.shape == (E,) and int(counts.sum()) == n_tok
    offsets = np.zeros(E, dtype=np.int64)
    offsets[1:] = np.cumsum(counts)[:-1]

    nc = _get_program(CAP)

    # Work list: split each expert's contiguous token segment into slots of
    # <= CAP tokens; process 8 slots per SPMD round. The uniform T/E = 1024
    # split is exactly one round of 8 slots.
    slots = []
    for e in range(E):
        cnt, off = int(counts[e]), int(offsets[e])
        for s in range(0, cnt, CAP):
            slots.append((e, off + s, min(CAP, cnt - s)))

    wuT_cache = {}
    wdT_cache = {}

    def expert_weights(e):
        if e not in wuT_cache:
            wuT_cache[e] = _swizzle_wu(w_up[e].astype(bf))
            wdT_cache[e] = _swizzle_wd(w_down[e].astype(bf))
        return wuT_cache[e], wdT_cache[e]

    out = np.zeros((n_tok, D), dtype=x.dtype)
    zero_map = None
    for r0 in range(0, len(slots), N_CORES):
        round_slots = slots[r0:r0 + N_CORES]
        in_maps = []
        for e, off, cnt in round_slots:
            xs = np.zeros((CAP, D), dtype=bf)
            xs[:cnt] = x[off:off + cnt].astype(bf)
            wuT, wdT = expert_weights(e)
            in_maps.append({
                "xT": np.ascontiguousarray(xs.T), "wuT": wuT, "wdT": wdT,
            })
        while len(in_maps) < N_CORES:  # idle cores in the last round
            if zero_map is None:
                zero_map = {
                    "xT": np.zeros((D, CAP), dtype=bf),
                    "wuT": np.zeros((P, H // P, n_d_host, P), dtype=bf),
                    "wdT": np.zeros((P, H // P, D), dtype=bf),
                }
            in_maps.append(zero_map)

        res = run_bass_kernel_spmd(
            nc, in_maps, core_ids=list(range(N_CORES)), trace=_trace
        )
        LAST_RESULT = res
        for i, (e, off, cnt) in enumerate(round_slots):
            out[off:off + cnt] = res.results[i]["y"][:cnt].astype(x.dtype)
    return out
